# revision 12
# baseline (speedup 1.0000x reference)
"""MoNet (2-layer GMMConv GNN) on 8 Trainium2 NeuronCores.

Strategy (edge-parallel with contiguous node ranges per core):
  - Nodes are split into 8 contiguous ranges of 6250 (core c owns nodes
    [c*6250, (c+1)*6250)).  Within a core, nodes are bin-packed into 49
    blocks of <=128 "slots" so each block's in-edge count is balanced.
  - Each core processes all edges whose dst lies in its range.  Edges are
    grouped per dst-block and split into a "lo" half (src < 25000) and a
    "hi" half (src >= 25000) so gather indices fit in int16 (dma_gather's
    index dtype); each half is padded to 9 chunks of 128 edge slots.
  - Per chunk, messages are aggregated into the block's 128 node slots via
    a one-hot selection matmul (S^T @ M accumulated in PSUM).  Layer-0
    gathers raw `feat` rows and applies Wfc0 *after* aggregation (the fc
    projection commutes with the weighted scatter-sum), so no hp table has
    to be materialized or replicated.
  - After layer 0, per-core h shards are AllGathered into a full slot-
    ordered h table, which layer 1 gathers from (same block/chunk layout).
  - Gaussian kernel weights (pseudo -> tanh -> exp) are computed on-device
    in bulk [128, 882] tiles from per-edge degree values.
All host-side work is pure integer index bookkeeping (bincount, grouping,
padding, int16 index image construction) plus input replication.
"""
import os

import numpy as np

os.environ.setdefault("JAX_PLATFORMS", "axon,cpu")

N = 50000
E = 800000
IN = 128
H = 64
OUT = 40
K = 3
P = 128
NCORES = 8
NPC = N // NCORES          # nodes per core (6250)
BPC = 56                   # blocks per core
SPC = BPC * P              # slots per core (7168)
CPH = 8                    # chunks per half (lo/hi); 8*128=1024 = max idxs
#                            a single dma_gather call supports
CPB = 2 * CPH              # chunks per block (18)
HALF = CPH * P             # edge slots per half (1152)
SPB = CPB * P              # edge slots per block (2304)
COLS = BPC * CPB           # gauss/dstloc columns per core (882)
SPLIT0 = N // 2            # feat table split (25000)
SPLIT1 = 4 * SPC           # h table split (25088)
NSLOT = NCORES * SPC       # total h-table rows (50176)
IDXW = HALF // 16          # int16 idx image cols per half (72)

_CACHE = {}


def _pack(src, dst):
    """Host-side integer preprocessing. Returns per-core input arrays and
    the slot->node mapping needed to unshard the output."""
    src = np.asarray(src).astype(np.int64)
    dst = np.asarray(dst).astype(np.int64)
    deg = np.bincount(dst, minlength=N)
    lo_cnt = np.bincount(dst[src < SPLIT0], minlength=N)
    hi_cnt = deg - lo_cnt

    block_of = np.empty(N, np.int64)    # global block id (core*BPC + b)
    slot_in_block = np.empty(N, np.int64)
    for c in range(NCORES):
        nodes = np.arange(c * NPC, (c + 1) * NPC)
        order = nodes[np.argsort(-deg[nodes], kind="stable")]
        loads_lo = np.zeros(BPC, np.int64)
        loads_hi = np.zeros(BPC, np.int64)
        counts = np.zeros(BPC, np.int64)
        for n in order:
            cost = np.maximum(loads_lo + lo_cnt[n], loads_hi + hi_cnt[n])
            cost[counts >= P] = 1 << 60
            b = int(np.argmin(cost))
            block_of[n] = c * BPC + b
            slot_in_block[n] = counts[b]
            counts[b] += 1
            loads_lo[b] += lo_cnt[n]
            loads_hi[b] += hi_cnt[n]
        assert loads_lo.max() <= HALF and loads_hi.max() <= HALF, (
            c, loads_lo.max(), loads_hi.max())
    gslot = block_of * P + slot_in_block          # global h-table row of node

    # --- per-edge slot assignment ---
    eb = block_of[dst]                            # global block of each edge
    eg = (src >= SPLIT0).astype(np.int64)         # half (0=lo, 1=hi)
    key = eb * 2 + eg
    order = np.argsort(key, kind="stable")
    ks = key[order]
    starts = np.searchsorted(ks, np.arange(2 * NCORES * BPC))
    gidx_in_group = np.empty(E, np.int64)
    gidx_in_group[order] = np.arange(E) - starts[ks]
    assert gidx_in_group.max() < HALF

    core_of_e = eb // BPC
    b_in_core = eb % BPC
    col = b_in_core * CPB + eg * CPH + gidx_in_group // P   # [0, COLS)
    part = gidx_in_group % P

    per_core = []
    unperm = gslot.copy()
    for c in range(NCORES):
        m = core_of_e == c
        pc, cc = part[m], col[m]
        dstloc = np.full((P, COLS), 200.0, np.float32)
        dsrc = np.ones((P, COLS), np.float32)
        ddst = np.ones((P, COLS), np.float32)
        dstloc[pc, cc] = slot_in_block[dst[m]]
        dsrc[pc, cc] = deg[src[m]]
        ddst[pc, cc] = deg[dst[m]]

        # int16 gather index images: [128, BPC*2*IDXW] (block, half, wrap16)
        def idx_image(values_full):
            img = np.zeros((16, BPC * 2 * IDXW), np.int16)
            gi = gidx_in_group[m]
            flat_col = (b_in_core[m] * 2 + eg[m]) * IDXW + gi // 16
            img[gi % 16, flat_col] = values_full
            return np.tile(img, (8, 1))

        sm = src[m]
        v0 = np.where(sm < SPLIT0, sm, sm - SPLIT0).astype(np.int16)
        gs = gslot[sm]
        v1 = np.where(sm < SPLIT0, gs, gs - SPLIT1).astype(np.int16)
        per_core.append(dict(
            dstloc=dstloc, dsrc=dsrc, ddst=ddst,
            gidx0=idx_image(v0), gidx1=idx_image(v1),
        ))
    return per_core, unperm


def _build():
    import concourse.bacc as bacc
    import concourse.bass as bass
    import concourse.tile as tile
    from concourse import mybir
    from concourse.masks import make_identity

    dt = mybir.dt
    nc = bacc.Bacc(None, target_bir_lowering=False)

    feat = nc.declare_dram_parameter("feat", [N, IN], dt.float32, isOutput=False)
    wfc0 = nc.declare_dram_parameter("wfc0", [IN, K * H], dt.float32, isOutput=False)
    wfc1 = nc.declare_dram_parameter("wfc1", [H, K * OUT], dt.float32, isOutput=False)
    # aux: [128, 128 iota | 64 b0 | 40 b1 | 2x18 coefs] f32, all replicated
    # down partitions except iota (row j = j at col j).
    AUXW = 128 + H + OUT + 36
    aux = nc.declare_dram_parameter("aux", [P, AUXW], dt.float32, isOutput=False)
    dstloc = nc.declare_dram_parameter("dstloc", [P, COLS], dt.float32, isOutput=False)
    dsrc = nc.declare_dram_parameter("dsrc", [P, COLS], dt.float32, isOutput=False)
    ddst = nc.declare_dram_parameter("ddst", [P, COLS], dt.float32, isOutput=False)
    gidx0 = nc.declare_dram_parameter("gidx0", [P, BPC * 2 * IDXW], dt.int16, isOutput=False)
    gidx1 = nc.declare_dram_parameter("gidx1", [P, BPC * 2 * IDXW], dt.int16, isOutput=False)
    y = nc.declare_dram_parameter("y", [SPC, OUT], dt.float32, isOutput=True)

    h_shard = nc.dram_tensor("h_shard", [SPC, H], dt.float32)
    h_full = nc.dram_tensor("h_full", [NSLOT, H], dt.float32, addr_space="Shared")

    IOTA0 = 0
    B0 = 128
    B1 = 128 + H
    CF = 128 + H + OUT  # coefs: per layer 18: wp00 wp10 bp0 wp01 wp11 bp1
    #                     then per k: mu_k0 isig_k0 mu_k1 isig_k1 (12)

    def cf(layer, i):
        return aux_t[:, CF + 18 * layer + i: CF + 18 * layer + i + 1]

    with tile.TileContext(nc) as tc:
        with (
            tc.tile_pool(name="cst", bufs=1) as cst,
            tc.tile_pool(name="gau", bufs=1) as gau,
            tc.tile_pool(name="sb", bufs=2) as sb,
            tc.tile_pool(name="ps", bufs=2, space="PSUM") as ps,
            tc.tile_pool(name="dram", bufs=1, space="DRAM") as dram,
        ):
            # ---- constant loads ----
            aux_t = cst.tile([P, AUXW], dt.float32)
            nc.sync.dma_start(out=aux_t[:], in_=aux[:])
            w0_t = cst.tile([P, COLS], dt.float32, tag="ldtmp")
            nc.sync.dma_start(out=w0_t[:IN, :K * H], in_=wfc0[:])
            w0r = cst.tile([IN, K * H], dt.float32r)
            nc.vector.tensor_copy(out=w0r[:], in_=w0_t[:IN, :K * H])
            w1_t = cst.tile([P, COLS], dt.float32, tag="ldtmp")
            nc.sync.dma_start(out=w1_t[:H, :K * OUT], in_=wfc1[:])
            w1r = cst.tile([H, K * OUT], dt.float32r)
            nc.vector.tensor_copy(out=w1r[:], in_=w1_t[:H, :K * OUT])
            ident = cst.tile([P, P], dt.float32)
            make_identity(nc, ident[:])
            dl_t = cst.tile([P, COLS], dt.float32)
            nc.sync.dma_start(out=dl_t[:], in_=dstloc[:])
            dl2 = cst.tile([P, COLS * 2], dt.bfloat16)
            nc.vector.tensor_copy(
                out=dl2[:].rearrange("p (c t) -> p c t", t=2),
                in_=dl_t[:][:, :, None].broadcast_to([P, COLS, 2]))
            io_bf = cst.tile([P, P], dt.bfloat16)
            nc.scalar.activation(out=io_bf[:], in_=aux_t[:, IOTA0:IOTA0 + P],
                                 func=mybir.ActivationFunctionType.Copy)

            # inverse sqrt degrees (shared by both layers)
            ds_t = cst.tile([P, COLS], dt.float32, tag="ldtmp")
            nc.sync.dma_start(out=ds_t[:], in_=dsrc[:])
            isd_s = gau.tile([P, COLS], dt.float32)
            nc.scalar.sqrt(out=isd_s[:], in_=ds_t[:])
            nc.vector.reciprocal(out=isd_s[:], in_=isd_s[:])
            dd_t = cst.tile([P, COLS], dt.float32, tag="ldtmp")
            nc.sync.dma_start(out=dd_t[:], in_=ddst[:])
            isd_d = gau.tile([P, COLS], dt.float32)
            nc.scalar.sqrt(out=isd_d[:], in_=dd_t[:])
            nc.vector.reciprocal(out=isd_d[:], in_=isd_d[:])
            gi0_t = cst.tile([P, BPC * 2 * IDXW], dt.int16, tag="gidx")
            nc.sync.dma_start(out=gi0_t[:], in_=gidx0[:])

            def gauss(layer):
                """per-edge gaussian kernel weights -> [P, COLS] tiles g0..g2"""
                gs = []
                pds = []
                for d in range(2):
                    m1 = gau.tile([P, COLS], dt.float32, tag="ta")
                    nc.vector.tensor_scalar(
                        out=m1[:], in0=isd_s[:], scalar1=cf(layer, 0 + 3 * d),
                        scalar2=None, op0=mybir.AluOpType.mult)
                    m2 = gau.tile([P, COLS], dt.float32, tag="tb")
                    nc.vector.tensor_scalar(
                        out=m2[:], in0=isd_d[:], scalar1=cf(layer, 1 + 3 * d),
                        scalar2=None, op0=mybir.AluOpType.mult)
                    nc.vector.tensor_tensor(
                        out=m1[:], in0=m1[:], in1=m2[:], op=mybir.AluOpType.add)
                    pd = gau.tile([P, COLS], dt.float32, tag=f"pd{d}")
                    nc.scalar.activation(
                        out=pd[:], in_=m1[:], func=mybir.ActivationFunctionType.Tanh,
                        bias=cf(layer, 2 + 3 * d))
                    pds.append(pd)
                for k in range(K):
                    a = gau.tile([P, COLS], dt.float32, tag="ta")
                    nc.vector.tensor_scalar(
                        out=a[:], in0=pds[0][:], scalar1=cf(layer, 6 + 4 * k),
                        scalar2=cf(layer, 7 + 4 * k),
                        op0=mybir.AluOpType.subtract, op1=mybir.AluOpType.mult)
                    nc.scalar.square(out=a[:], in_=a[:])
                    b = gau.tile([P, COLS], dt.float32, tag="tb")
                    nc.vector.tensor_scalar(
                        out=b[:], in0=pds[1][:], scalar1=cf(layer, 8 + 4 * k),
                        scalar2=cf(layer, 9 + 4 * k),
                        op0=mybir.AluOpType.subtract, op1=mybir.AluOpType.mult)
                    nc.scalar.square(out=b[:], in_=b[:])
                    nc.vector.tensor_tensor(
                        out=a[:], in0=a[:], in1=b[:], op=mybir.AluOpType.add)
                    gt_ = gau.tile([P, COLS], dt.bfloat16, tag="tb")
                    nc.scalar.activation(
                        out=gt_[:], in_=a[:], func=mybir.ActivationFunctionType.Exp,
                        scale=-0.5)
                    g2 = gau.tile([P, COLS * 2], dt.bfloat16, tag=f"g{k}")
                    nc.vector.tensor_copy(
                        out=g2[:].rearrange("p (c t) -> p c t", t=2),
                        in_=gt_[:][:, :, None].broadcast_to([P, COLS, 2]))
                    gs.append(g2)
                return gs

            g0 = gauss(0)

            # ---------------- layer 0 ----------------
            for b in range(BPC):
                gt = sb.tile([P, CPB, IN], dt.float32, tag="gath")
                ib = b * 2 * IDXW
                nc.gpsimd.dma_gather(
                    out_ap=gt[:, :CPH, :], in_ap=feat[:SPLIT0],
                    idxs_ap=gi0_t[:, ib: ib + IDXW],
                    num_idxs=HALF, num_idxs_reg=HALF, elem_size=IN)
                nc.gpsimd.dma_gather(
                    out_ap=gt[:, CPH:, :], in_ap=feat[SPLIT0:],
                    idxs_ap=gi0_t[:, ib + IDXW: ib + 2 * IDXW],
                    num_idxs=HALF, num_idxs_reg=HALF, elem_size=IN)

                cs = slice(b * CPB, (b + 1) * CPB)
                gtb = sb.tile([P, CPB, IN], dt.bfloat16, tag="gathbf")
                nc.scalar.activation(out=gtb[:], in_=gt[:],
                                     func=mybir.ActivationFunctionType.Copy)
                s_blk = sb.tile([P, CPB, P], dt.bfloat16, tag="onehot")
                nc.vector.tensor_tensor(
                    out=s_blk[:].rearrange("p c (j t) -> p c j t", t=2),
                    in0=io_bf[:][:, None, :].broadcast_to([P, CPB, P])
                        .rearrange("p c (j t) -> p c j t", t=2),
                    in1=dl2[:].rearrange("p (c t) -> p c t", t=2)[:, cs, :]
                        [:, :, None, :].broadcast_to([P, CPB, P // 2, 2]),
                    op=mybir.AluOpType.is_equal)
                scl = sb.tile([P, K, CPB * IN], dt.bfloat16, tag="scaled")
                for k in range(K):
                    nc.vector.tensor_tensor(
                        out=scl[:, k, :].rearrange(
                            "p (c i t) -> p c i t", c=CPB, t=2),
                        in0=gtb[:].rearrange("p c (i t) -> p c i t", t=2),
                        in1=g0[k][:].rearrange("p (c t) -> p c t", t=2)[:, cs, :]
                            [:, :, None, :].broadcast_to([P, CPB, IN // 2, 2]),
                        op=mybir.AluOpType.mult)

                acc = ps.tile([P, K * IN], dt.float32, tag="agg")
                scl3 = scl[:].rearrange("p k (c i) -> p k c i", i=IN)
                for c in range(CPB):
                    nc.tensor.matmul(
                        out=acc[:], lhsT=s_blk[:, c, :], rhs=scl3[:, :, c, :],
                        start=(c == 0), stop=(c == CPB - 1))

                pre = sb.tile([P, K * IN], dt.float32, tag="pre")
                nc.scalar.activation(
                    out=pre[:], in_=acc[:], func=mybir.ActivationFunctionType.Copy)
                hp = ps.tile([P, H], dt.float32, tag="post")
                for k in range(K):
                    tp = ps.tile([P, P], dt.float32, tag="tp")
                    nc.tensor.transpose(
                        out=tp[:], in_=pre[:, k * IN:(k + 1) * IN], identity=ident[:])
                    tps = sb.tile([P, P], dt.float32r, tag="tps")
                    nc.vector.tensor_copy(out=tps[:], in_=tp[:])
                    nc.tensor.matmul(
                        out=hp[:], lhsT=tps[:], rhs=w0r[:, k * H:(k + 1) * H],
                        start=(k == 0), stop=(k == K - 1))
                h_sb = sb.tile([P, H], dt.float32, tag="outt")
                nc.vector.tensor_tensor(
                    out=h_sb[:], in0=hp[:], in1=aux_t[:, B0:B0 + H],
                    op=mybir.AluOpType.add)
                nc.sync.dma_start(out=h_shard[b * P:(b + 1) * P, :], in_=h_sb[:])

            gi1_t = cst.tile([P, BPC * 2 * IDXW], dt.int16, tag="gidx")
            nc.sync.dma_start(out=gi1_t[:], in_=gidx1[:])

            # ---------------- allgather h ----------------
            nc.gpsimd.collective_compute(
                "AllGather", mybir.AluOpType.bypass,
                replica_groups=[list(range(NCORES))],
                ins=[h_shard[:]], outs=[h_full[:]])

            g1 = gauss(1)

            # ---------------- layer 1 ----------------
            for b in range(BPC):
                gt = sb.tile([P, CPB, H], dt.float32, tag="gath")
                ib = b * 2 * IDXW
                nc.gpsimd.dma_gather(
                    out_ap=gt[:, :CPH, :], in_ap=h_full[:SPLIT1],
                    idxs_ap=gi1_t[:, ib: ib + IDXW],
                    num_idxs=HALF, num_idxs_reg=HALF, elem_size=H)
                nc.gpsimd.dma_gather(
                    out_ap=gt[:, CPH:, :], in_ap=h_full[SPLIT1:],
                    idxs_ap=gi1_t[:, ib + IDXW: ib + 2 * IDXW],
                    num_idxs=HALF, num_idxs_reg=HALF, elem_size=H)

                cs = slice(b * CPB, (b + 1) * CPB)
                gtb = sb.tile([P, CPB, H], dt.bfloat16, tag="gathbf")
                nc.scalar.activation(out=gtb[:], in_=gt[:],
                                     func=mybir.ActivationFunctionType.Copy)
                s_blk = sb.tile([P, CPB, P], dt.bfloat16, tag="onehot")
                nc.vector.tensor_tensor(
                    out=s_blk[:].rearrange("p c (j t) -> p c j t", t=2),
                    in0=io_bf[:][:, None, :].broadcast_to([P, CPB, P])
                        .rearrange("p c (j t) -> p c j t", t=2),
                    in1=dl2[:].rearrange("p (c t) -> p c t", t=2)[:, cs, :]
                        [:, :, None, :].broadcast_to([P, CPB, P // 2, 2]),
                    op=mybir.AluOpType.is_equal)
                scl = sb.tile([P, K, CPB * H], dt.bfloat16, tag="scaled")
                for k in range(K):
                    nc.vector.tensor_tensor(
                        out=scl[:, k, :].rearrange(
                            "p (c i t) -> p c i t", c=CPB, t=2),
                        in0=gtb[:].rearrange("p c (i t) -> p c i t", t=2),
                        in1=g1[k][:].rearrange("p (c t) -> p c t", t=2)[:, cs, :]
                            [:, :, None, :].broadcast_to([P, CPB, H // 2, 2]),
                        op=mybir.AluOpType.mult)

                acc = ps.tile([P, K * H], dt.float32, tag="agg")
                scl3 = scl[:].rearrange("p k (c i) -> p k c i", i=H)
                for c in range(CPB):
                    nc.tensor.matmul(
                        out=acc[:], lhsT=s_blk[:, c, :], rhs=scl3[:, :, c, :],
                        start=(c == 0), stop=(c == CPB - 1))

                pre = sb.tile([P, K * H], dt.float32, tag="pre")
                nc.scalar.activation(
                    out=pre[:], in_=acc[:], func=mybir.ActivationFunctionType.Copy)
                yp = ps.tile([P, OUT], dt.float32, tag="post")
                for k in range(K):
                    tp = ps.tile([H, P], dt.float32, tag="tp")
                    nc.tensor.transpose(
                        out=tp[:], in_=pre[:, k * H:(k + 1) * H], identity=ident[:])
                    tps = sb.tile([H, P], dt.float32r, tag="tps")
                    nc.vector.tensor_copy(out=tps[:], in_=tp[:])
                    nc.tensor.matmul(
                        out=yp[:], lhsT=tps[:], rhs=w1r[:, k * OUT:(k + 1) * OUT],
                        start=(k == 0), stop=(k == K - 1))
                y_sb = sb.tile([P, OUT], dt.float32, tag="outt")
                nc.vector.tensor_tensor(
                    out=y_sb[:], in0=yp[:], in1=aux_t[:, B1:B1 + OUT],
                    op=mybir.AluOpType.add)
                nc.sync.dma_start(out=y[b * P:(b + 1) * P, :], in_=y_sb[:])

    nc.finalize()
    return nc


def _aux_array(Wp0, bp0, mu0, isig0, b0, Wp1, bp1, mu1, isig1, b1):
    AUXW = 128 + H + OUT + 36
    aux = np.zeros((P, AUXW), np.float32)
    aux[:, :128] = np.arange(128, dtype=np.float32)[None, :]
    aux[:, 128:128 + H] = np.asarray(b0, np.float32)[None, :]
    aux[:, 128 + H:128 + H + OUT] = np.asarray(b1, np.float32)[None, :]
    base = 128 + H + OUT
    for li, (Wp, bp, mu, isig) in enumerate(
            [(Wp0, bp0, mu0, isig0), (Wp1, bp1, mu1, isig1)]):
        Wp = np.asarray(Wp, np.float32)
        bp = np.asarray(bp, np.float32)
        mu = np.asarray(mu, np.float32)
        isig = np.asarray(isig, np.float32)
        cfv = np.empty(18, np.float32)
        for d in range(2):
            cfv[3 * d + 0] = Wp[0, d]
            cfv[3 * d + 1] = Wp[1, d]
            cfv[3 * d + 2] = bp[d]
        for k in range(K):
            cfv[6 + 4 * k + 0] = mu[k, 0]
            cfv[6 + 4 * k + 1] = isig[k, 0]
            cfv[6 + 4 * k + 2] = mu[k, 1]
            cfv[6 + 4 * k + 3] = isig[k, 1]
        aux[:, base + 18 * li: base + 18 * (li + 1)] = cfv[None, :]
    return aux


def kernel(feat, src, dst,
           Wp0, bp0, mu0, isig0, Wfc0, b0,
           Wp1, bp1, mu1, isig1, Wfc1, b1,
           _trace=False):
    from concourse.bass_utils import run_bass_kernel_spmd

    feat = np.ascontiguousarray(np.asarray(feat, np.float32))
    src_i = np.asarray(src)
    dst_i = np.asarray(dst)

    pk = _CACHE.get("pack")
    if pk is None or not (np.array_equal(_CACHE["src"], src_i)
                          and np.array_equal(_CACHE["dst"], dst_i)):
        pk = _pack(src_i, dst_i)
        _CACHE["pack"] = pk
        _CACHE["src"] = np.asarray(src_i).copy()
        _CACHE["dst"] = np.asarray(dst_i).copy()
    per_core, gslot = pk

    nc = _CACHE.get("nc")
    if nc is None:
        nc = _build()
        _CACHE["nc"] = nc

    aux = _aux_array(Wp0, bp0, mu0, isig0, b0, Wp1, bp1, mu1, isig1, b1)
    wfc0 = np.ascontiguousarray(np.asarray(Wfc0, np.float32))
    wfc1 = np.ascontiguousarray(np.asarray(Wfc1, np.float32))
    in_maps = []
    for c in range(NCORES):
        d = per_core[c]
        in_maps.append(dict(
            feat=feat, wfc0=wfc0, wfc1=wfc1, aux=aux,
            dstloc=d["dstloc"], dsrc=d["dsrc"], ddst=d["ddst"],
            gidx0=d["gidx0"], gidx1=d["gidx1"],
        ))

    res = run_bass_kernel_spmd(nc, in_maps, list(range(NCORES)),
                               trace=_trace)
    shards = np.stack([res.results[c]["y"] for c in range(NCORES)], axis=0)
    full = shards.reshape(NCORES * SPC, OUT)
    out = full[gslot]          # gslot[n] = h-table/slot row of node n
    if _trace:
        return out, res
    return out
